# revision 57
# baseline (speedup 1.0000x reference)
"""MultiHeadDistanceKNN Trainium2 kernel.

kernel(x, W) -> adj : x [2,2048,512] f32, W [4,512,128] f32 -> adj [2,2048,2048] f32.

8 cores = 4 heads x 2 batches; core i handles (h=i//2, b=i%2) and computes
  C[n,m] = exp(-d2[n,m]/(2*mu^2)) * 1[d2[n,m] <= max(T_n, T_m)]
where d2 = |z_n - z_m|^2 for z = x_b @ W_h (exactly symmetric), T_n = K-th
smallest of row n (K=307), mu = mean distance. Host: adj[b] = mean_h C.

d2 is produced entirely on the PE: psum = (-2 zT)^T zT + [sq;1]^T [1;sq]
(K=4 augmented matmul), then ACT relu-copies psum -> SBUF-resident D2.
T_n: per-row Gaussian bracket (mu/sigma from an in-build 1/4-subsample
centered variance on ACT; counts at lo/hi fused into the build on DVE),
then 4 secant rounds (Illinois damping, counts split DVE 8 / ACT 8 with
per-engine scratch pools), then exact top-8-below-hi finisher
(W=(d2<=hi)*d2 stt + vector.max) picking T = v_K when chi-K in [0,7].
Device emits A = sim * (d2 <= T_n) rowmask in bf16; the host computes
C = max(A, A^T) (exact: sim symmetric, >=0) and the 4-head mean.
"""
import numpy as np

import concourse.bass as bass
import concourse.mybir as mybir
from concourse import bacc
from concourse.tile import TileContext
from concourse.masks import make_identity

F32 = mybir.dt.float32
BF16 = mybir.dt.bfloat16
F8E5 = mybir.dt.float8e5
U8 = mybir.dt.uint8
DROW = mybir.MatmulPerfMode.DoubleRow
Alu = mybir.AluOpType
Act = mybir.ActivationFunctionType
X_AX = mybir.AxisListType.X
XY_AX = mybir.AxisListType.XY

N = 2048
D = 512
NCH = 16
NJT = 4
K = 307

N_ROUNDS = 4          # round 0 is the fused fixed-phi probe
AIM = 1.5
PHI = 0.55
ZLO = -1.19
ZHI = -0.85
DVE_CNT = 8           # count chunks on DVE per round (rest on ACT)


def build_nc():
    nc = bacc.Bacc("TRN2", target_bir_lowering=False)
    xb = nc.dram_tensor("xb", [N, D], F32, kind="ExternalInput")
    wh = nc.dram_tensor("wh", [D, 128], F32, kind="ExternalInput")
    outp = nc.dram_tensor("outp", [N, N], BF16, kind="ExternalOutput")

    with TileContext(nc) as tc:
        with tc.tile_pool(name="base", bufs=1) as base, \
             tc.tile_pool(name="st", bufs=1) as st:
            D2 = base.tile([128, NCH * N], F32)
            zT = base.tile([128, N], F32)
            ident = base.tile([128, 128], F32)
            make_identity(nc, ident[:])
            ones_col = base.tile([128, 1], F32)
            nc.gpsimd.memset(ones_col[:], 1.0)
            ones_row = base.tile([1, 128], F32)
            nc.gpsimd.memset(ones_row[:], 1.0)
            id1 = base.tile([1, 1], F32)
            nc.gpsimd.memset(id1[:], 1.0)

            def stt16(name, w=NCH, dt=F32):
                return st.tile([128, w], dt, tag=name, name=name)
            sqcol = stt16("sqcol"); zdots = stt16("zdots")
            mu = stt16("mu"); negmu = stt16("negmu")
            vacc = stt16("vacc"); s16 = stt16("s16")
            neglo = stt16("neglo")
            width0 = stt16("width0")
            # paired state, ping-pong: [:, 0:16] = value, [:, 16:32] = count
            lc = [stt16("lcA", 32), stt16("lcB", 32)]     # (lo, clo)
            hc = [stt16("hcA", 32), stt16("hcB", 32)]     # (hi, chi)
            chieff = [stt16("ceA"), stt16("ceB")]
            tcnt = stt16("tcnt", 32)                       # (t, cnt)
            negt = stt16("negt"); sacc = stt16("sacc")
            musum = stt16("musum")
            Tfin = stt16("Tfin")
            tmp1 = stt16("tmp1"); tmp2 = stt16("tmp2"); tmp3 = stt16("tmp3")
            tmp4 = stt16("tmp4")
            mge = stt16("mge", NCH, U8)
            mbl = stt16("mbl", NCH, U8)
            mbh = stt16("mbh", NCH, U8)
            mok = stt16("mok", NCH, U8)
            mge32 = stt16("mge32", 32, U8)
            mbl32 = stt16("mbl32", 32, U8)
            mbh32 = stt16("mbh32", 32, U8)
            s_vec = st.tile([128, 1], F32, tag="s_vec", name="s_vec")
            s1b = st.tile([128, 1], F32, tag="s1b", name="s1b")
            neginvb = st.tile([128, 1], F32, tag="neginvb", name="neginvb")
            sc1 = st.tile([1, 1], F32, tag="sc1", name="sc1")
            sc2 = st.tile([1, 1], F32, tag="sc2", name="sc2")
            sc3 = st.tile([1, 1], F32, tag="sc3", name="sc3")
            sc4 = st.tile([1, 1], F32, tag="sc4", name="sc4")
            w8 = st.tile([128, NCH * 8], F32, tag="w8", name="w8")
            iota8f = st.tile([128, NCH * 8], F32, tag="iota8f", name="iota8f")
            nc.gpsimd.iota(iota8f[:], pattern=[[0, NCH], [1, 8]], base=0,
                           channel_multiplier=0,
                           allow_small_or_imprecise_dtypes=True)
            ohsel = st.tile([128, NCH * 8], F32, tag="ohsel", name="ohsel")

            # ================= prep + build + mu =================
            with tc.tile_pool(name="mid", bufs=1) as mid:
                sqrow = mid.tile([1, N], F32, tag="sqrow", name="sqrow")

                with tc.tile_pool(name="prep", bufs=3) as prep, \
                     tc.tile_pool(name="prep1", bufs=1) as prep1, \
                     tc.tile_pool(name="pps", bufs=3, space="PSUM") as pps, \
                     tc.tile_pool(name="pps1", bufs=4, space="PSUM") as pps1:
                    w_sb = prep1.tile([128, D], F32)
                    for dc in range(4):
                        nc.gpsimd.dma_start(w_sb[:, dc * 128:(dc + 1) * 128],
                                            wh[dc * 128:(dc + 1) * 128, :])
                    xt = [prep1.tile([128, N], F32, tag=f"xt{dc}", name=f"xt{dc}")
                          for dc in range(4)]
                    for c in range(NCH):
                        x_sb = prep.tile([128, D], F32, tag="x_sb", name="x_sb")
                        nc.gpsimd.dma_start(x_sb[:], xb[c * 128:(c + 1) * 128, :])
                        for dc in range(4):
                            tr_ps = pps1.tile([128, 128], F32, tag="small",
                                              name="tr")
                            nc.tensor.transpose(
                                tr_ps[:], x_sb[:, dc * 128:(dc + 1) * 128],
                                ident[:])
                            if dc % 2 == 0:
                                nc.scalar.copy(
                                    xt[dc][:, c * 128:(c + 1) * 128], tr_ps[:])
                            else:
                                nc.vector.tensor_copy(
                                    xt[dc][:, c * 128:(c + 1) * 128], tr_ps[:])
                    tc.strict_bb_all_engine_barrier()
                    for j in range(NJT):
                        zt_ps = pps.tile([128, 512], F32, tag="big", name="zt")
                        for dc in range(4):
                            nc.tensor.matmul(
                                zt_ps[:], w_sb[:, dc * 128:(dc + 1) * 128],
                                xt[dc][:, j * 512:(j + 1) * 512],
                                start=(dc == 0), stop=(dc == 3))
                        nc.vector.tensor_copy(zT[:, j * 512:(j + 1) * 512],
                                              zt_ps[:])
                    # squared norms -> sqrow, sqcol
                    zT2 = D2[:, 0:N]
                    nc.scalar.activation(zT2, zT[:], Act.Square)
                    for j in range(NJT):
                        sq_ps = pps1.tile([1, 512], F32, tag="small", name="sqps")
                        nc.tensor.matmul(sq_ps[:], ones_col[:],
                                         zT2[:, j * 512:(j + 1) * 512],
                                         start=True, stop=True)
                        nc.vector.tensor_copy(sqrow[0:1, j * 512:(j + 1) * 512],
                                              sq_ps[:])
                    for c in range(NCH):
                        tp = pps1.tile([128, 1], F32, tag="small", name="sqcolp")
                        nc.tensor.transpose(tp[:],
                                            sqrow[0:1, c * 128:(c + 1) * 128],
                                            id1[:])
                        nc.scalar.copy(sqcol[:, c:c + 1], tp[:])
                    nc.vector.tensor_reduce(s_vec[:], zT[:], axis=X_AX,
                                            op=Alu.add)
                    for c in range(NCH):
                        zd_ps = pps1.tile([128, 1], F32, tag="small", name="zdps")
                        nc.tensor.matmul(zd_ps[:], zT[:, c * 128:(c + 1) * 128],
                                         s_vec[:], start=True, stop=True)
                        nc.scalar.copy(zdots[:, c:c + 1], zd_ps[:])
                    nc.vector.tensor_reduce(sc1[:], sqrow[0:1, :], axis=X_AX,
                                            op=Alu.add)
                    s1_ps = pps1.tile([128, 1], F32, tag="small", name="s1ps")
                    nc.tensor.matmul(s1_ps[:], ones_row[:], sc1[:],
                                     start=True, stop=True)
                    nc.scalar.activation(s1b[:], s1_ps[:], Act.Copy,
                                         scale=1.0 / N)

                    nc.vector.scalar_tensor_tensor(
                        out=mu[:], in0=zdots[:], scalar=-2.0 / N, in1=sqcol[:],
                        op0=Alu.mult, op1=Alu.add)
                    nc.vector.tensor_scalar(mu[:], mu[:], s1b[:], scalar2=None,
                                            op0=Alu.add)
                    nc.vector.tensor_scalar(negmu[:], mu[:], -1.0,
                                            scalar2=None, op0=Alu.mult)

                # ---------------- build: d2 fully on PE ----------------
                with tc.tile_pool(name="bld", bufs=1) as bld, \
                     tc.tile_pool(name="bscr", bufs=1) as bscr, \
                     tc.tile_pool(name="sqs", bufs=1) as sqsp, \
                     tc.tile_pool(name="bps", bufs=6, space="PSUM") as bps:
                    # bf16 split of z (rhs) and -2z (lhsT); K=4 bf16 aug for
                    # sq_n + sq_m. Dropped term zl*zl' ~ 1e-4 << knn gaps.
                    zh = bld.tile([128, N], BF16, tag="zh", name="zh")
                    zl = bld.tile([128, N], BF16, tag="zl", name="zl")
                    vh = bld.tile([128, N], BF16, tag="vh", name="vh")
                    vl = bld.tile([128, N], BF16, tag="vl", name="vl")
                    nc.vector.tensor_copy(zh[:], zT[:])
                    nc.vector.tensor_sub(zl[:], zT[:], zh[:])
                    nc.vector.tensor_scalar(vh[:], zh[:], -2.0, scalar2=None,
                                            op0=Alu.mult)
                    nc.vector.tensor_scalar(vl[:], zl[:], -2.0, scalar2=None,
                                            op0=Alu.mult)
                    aug_a = bld.tile([4, N], BF16, tag="aug_a", name="aug_a")
                    aug_b = bld.tile([4, N], BF16, tag="aug_b", name="aug_b")
                    ones1n = bld.tile([1, N], BF16, tag="ones1n", name="ones1n")
                    sqh1 = bld.tile([1, N], BF16, tag="sqh1", name="sqh1")
                    sql1 = bld.tile([1, N], BF16, tag="sql1", name="sql1")
                    nc.gpsimd.memset(ones1n[:], 1.0)
                    nc.vector.tensor_copy(sqh1[0:1, :], sqrow[0:1, :])
                    nc.vector.tensor_sub(sql1[0:1, :], sqrow[0:1, :], sqh1[0:1, :])
                    nc.vector.tensor_copy(aug_a[0:1, :], sqh1[0:1, :])
                    nc.sync.dma_start(aug_a[1:2, :], sql1[0:1, :])
                    nc.sync.dma_start(aug_a[2:3, :], ones1n[0:1, :])
                    nc.sync.dma_start(aug_a[3:4, :], ones1n[0:1, :])
                    nc.vector.tensor_copy(aug_b[0:1, :], ones1n[0:1, :])
                    nc.sync.dma_start(aug_b[1:2, :], ones1n[0:1, :])
                    nc.sync.dma_start(aug_b[2:3, :], sqh1[0:1, :])
                    nc.sync.dma_start(aug_b[3:4, :], sql1[0:1, :])
                    tc.strict_bb_all_engine_barrier()
                    for c in range(NCH):
                        d2c = D2[:, c * N:(c + 1) * N]
                        cs = slice(c * 128, (c + 1) * 128)
                        for j in range(NJT):
                            js = slice(j * 512, (j + 1) * 512)
                            zz_ps = bps.tile([128, 512], F32, tag="zz",
                                             name="zz")
                            nc.tensor.matmul(zz_ps[:], vh[:, cs], zh[:, js],
                                             start=True, stop=False)
                            nc.tensor.matmul(zz_ps[:], vh[:, cs], zl[:, js],
                                             start=False, stop=False)
                            nc.tensor.matmul(zz_ps[:], vl[:, cs], zh[:, js],
                                             start=False, stop=False)
                            nc.tensor.matmul(zz_ps[:], aug_a[:, cs],
                                             aug_b[:, js],
                                             start=False, stop=True)
                            nc.scalar.activation(
                                d2c[:, js], zz_ps[:], Act.Relu)
                        sub = d2c.rearrange("p (a b) -> p a b", b=4)[:, :, 0:1]
                        sqs = sqsp.tile([128, 512], F32, tag="sqs", name="sqs")
                        nc.scalar.activation(
                            sqs[:], sub, Act.Sqrt, accum_out=musum[:, c:c + 1])
                        # row variance from the same 1/4 subsample (ACT,
                        # PE-shadowed): sum (d2 - mu)^2
                        vscr = sqsp.tile([128, 512], F32, tag="sqs",
                                         name="vscr")
                        nc.scalar.activation(
                            vscr[:], sub, Act.Square, bias=negmu[:, c:c + 1],
                            accum_out=vacc[:, c:c + 1])
                        # s = sqrt(v/512 + 1); +1 guards degenerate rows
                        nc.scalar.activation(s16[:, c:c + 1],
                                             vacc[:, c:c + 1], Act.Sqrt,
                                             bias=1.0, scale=1.0 / 512.0)
                        # per-row bracket lo/hi = mu + z*s
                        nc.vector.scalar_tensor_tensor(
                            out=lc[0][:, c:c + 1], in0=s16[:, c:c + 1],
                            scalar=ZLO, in1=mu[:, c:c + 1],
                            op0=Alu.mult, op1=Alu.add)
                        nc.vector.scalar_tensor_tensor(
                            out=hc[0][:, c:c + 1], in0=s16[:, c:c + 1],
                            scalar=ZHI, in1=mu[:, c:c + 1],
                            op0=Alu.mult, op1=Alu.add)
                        # fused round0 counts at hi and lo (DVE, PE-shadowed)
                        cscr = bscr.tile([128, N], F32, tag="cscr", name="cscr")
                        nc.vector.tensor_scalar(
                            cscr[:], d2c, hc[0][:, c:c + 1], scalar2=None,
                            op0=Alu.is_le, op1=Alu.add,
                            accum_out=hc[0][:, NCH + c:NCH + c + 1])
                        cscr2 = bscr.tile([128, N], F32, tag="cscr",
                                          name="cscr2")
                        nc.vector.tensor_scalar(
                            cscr2[:], d2c, lc[0][:, c:c + 1], scalar2=None,
                            op0=Alu.is_le, op1=Alu.add,
                            accum_out=lc[0][:, NCH + c:NCH + c + 1])
                    nc.vector.tensor_sub(width0[:], hc[0][:, 0:NCH],
                                         lc[0][:, 0:NCH])
                    nc.vector.tensor_copy(chieff[0][:], hc[0][:, NCH:32])

                # ---------------- mean distance ----------------
                with tc.tile_pool(name="mps", bufs=2, space="PSUM") as mps:
                    nc.vector.tensor_reduce(s_vec[:], musum[:], axis=X_AX,
                                            op=Alu.add)
                    ms_ps = mps.tile([1, 1], F32, tag="m", name="msps")
                    nc.tensor.matmul(ms_ps[:], ones_col[:], s_vec[:],
                                     start=True, stop=True)
                    nc.scalar.activation(sc2[:], ms_ps[:], Act.Copy,
                                         scale=1.0 / (N * 512.0))
                    nc.vector.tensor_reduce(
                        sc3[:],
                        sqrow[0:1, :].rearrange("p (a b) -> p a b", b=4)[:, :, 0:1],
                        axis=XY_AX, op=Alu.add)
                    nc.vector.tensor_scalar(sc3[:], sc3[:], 1.0 / 512.0,
                                            scalar2=None, op0=Alu.mult)
                    nc.vector.scalar_tensor_tensor(
                        out=sc4[:], in0=sc1[:], scalar=1.0 / N, in1=sc3[:],
                        op0=Alu.mult, op1=Alu.subtract)
                    nc.vector.tensor_scalar(sc3[:], sc2[:], 2.0, scalar2=None,
                                            op0=Alu.mult)
                    nc.vector.reciprocal(sc3[:], sc3[:])
                    nc.vector.tensor_mul(sc4[:], sc4[:], sc3[:])
                    nc.vector.tensor_add(sc2[:], sc2[:], sc4[:])
                    nc.vector.tensor_mul(sc2[:], sc2[:], sc2[:])
                    nc.vector.tensor_scalar(sc2[:], sc2[:], 2.0, scalar2=1e-8,
                                            op0=Alu.mult, op1=Alu.add)
                    nc.vector.reciprocal(sc2[:], sc2[:])
                    nc.vector.tensor_scalar(sc2[:], sc2[:], -1.0, scalar2=None,
                                            op0=Alu.mult)
                    ni_ps = mps.tile([128, 1], F32, tag="m", name="nips")
                    nc.tensor.matmul(ni_ps[:], ones_row[:], sc2[:],
                                     start=True, stop=True)
                    nc.vector.tensor_copy(neginvb[:], ni_ps[:])

            # ================= selection rounds =================
            # Separate scratch pools per engine: a shared pool serializes
            # ACT counting behind DVE counting via buffer-recycle deps.
            with tc.tile_pool(name="rscrA", bufs=2) as rscrA, \
                 tc.tile_pool(name="rscrB", bufs=2) as rscrB:
                for r in range(N_ROUNDS):
                    A, B = r % 2, (r + 1) % 2
                    loA, cloA = lc[A][:, 0:NCH], lc[A][:, NCH:32]
                    hiA, chiA = hc[A][:, 0:NCH], hc[A][:, NCH:32]
                    tA, cntA = tcnt[:, 0:NCH], tcnt[:, NCH:32]
                    # t = lo + clamp((K+AIM-clo)/(chieff-clo),.02,.98)*(hi-lo)
                    nc.vector.tensor_sub(tmp1[:], chieff[A][:], cloA)
                    nc.vector.tensor_scalar(tmp1[:], tmp1[:], 1.0,
                                            scalar2=None, op0=Alu.max)
                    nc.vector.reciprocal(tmp1[:], tmp1[:])
                    nc.vector.tensor_scalar(tmp2[:], cloA, -1.0,
                                            scalar2=float(K) + AIM,
                                            op0=Alu.mult, op1=Alu.add)
                    nc.vector.tensor_mul(tmp1[:], tmp1[:], tmp2[:])
                    nc.vector.tensor_scalar(tmp1[:], tmp1[:], 0.02,
                                            scalar2=0.98,
                                            op0=Alu.max, op1=Alu.min)
                    nc.vector.tensor_sub(tmp2[:], hiA, loA)
                    nc.vector.tensor_mul(tmp1[:], tmp1[:], tmp2[:])
                    nc.vector.tensor_add(tA, tmp1[:], loA)
                    # repair masks: chi<K -> probe above, clo>=K ->
                    # probe below (tight brackets, either can miss)
                    repair = r < 2
                    if repair:
                        nc.vector.tensor_scalar(mbh[:], chiA, float(K),
                                                scalar2=None,
                                                op0=Alu.is_lt)
                        nc.vector.scalar_tensor_tensor(
                            out=tmp3[:], in0=width0[:],
                            scalar=float(2.0 ** r),
                            in1=hiA, op0=Alu.mult, op1=Alu.add)
                        nc.vector.select(tA, mbh[:], tmp3[:], tA)
                        nc.vector.tensor_scalar(mbl[:], cloA, float(K),
                                                scalar2=None,
                                                op0=Alu.is_ge)
                        nc.vector.scalar_tensor_tensor(
                            out=tmp3[:], in0=width0[:],
                            scalar=-float(2.0 ** r),
                            in1=loA, op0=Alu.mult, op1=Alu.add)
                        nc.vector.select(tA, mbl[:], tmp3[:], tA)
                    nc.vector.tensor_scalar(negt[:], tA, -1.0,
                                            scalar2=None, op0=Alu.mult)
                    # counts
                    for c in range(NCH):
                        d2c = D2[:, c * N:(c + 1) * N]
                        if c < DVE_CNT:
                            scr = rscrA.tile([128, N], F32, tag="rscrA",
                                             name="rscrA")
                            nc.vector.tensor_scalar(
                                scr[:], d2c, tA[:, c:c + 1], scalar2=None,
                                op0=Alu.is_le, op1=Alu.add,
                                accum_out=cntA[:, c:c + 1])
                        else:
                            scr = rscrB.tile([128, N], F32, tag="rscrB",
                                             name="rscrB")
                            nc.scalar.activation(
                                scr[:], d2c, Act.Sign,
                                bias=negt[:, c:c + 1],
                                accum_out=sacc[:, c:c + 1])
                    if DVE_CNT < NCH:
                        nc.vector.tensor_scalar(
                            cntA[:, DVE_CNT:NCH], sacc[:, DVE_CNT:NCH],
                            -0.5, scalar2=float(N) * 0.5,
                            op0=Alu.mult, op1=Alu.add)
                    # updates -> B set
                    nc.vector.tensor_scalar(mge[:], cntA, float(K),
                                            scalar2=None, op0=Alu.is_ge)
                    masks = [(mge, mge32)] + (
                        [(mbh, mbh32), (mbl, mbl32)] if repair else [])
                    for msrc, mdst in masks:
                        nc.vector.tensor_copy(mdst[:, 0:NCH], msrc[:])
                        nc.vector.tensor_copy(mdst[:, NCH:32], msrc[:])
                    ge2 = mge32[:].rearrange("p (f c) -> p f c", f=2)
                    bh2 = mbh32[:].rearrange("p (f c) -> p f c", f=2)
                    bl2 = mbl32[:].rearrange("p (f c) -> p f c", f=2)

                    def pv(t):
                        return t[:].rearrange("p (f c) -> p f c", f=2)
                    nc.vector.select(pv(lc[B]), ge2, pv(lc[A]), pv(tcnt))
                    nc.vector.select(pv(hc[B]), ge2, pv(tcnt), pv(hc[A]))
                    if repair:
                        nc.vector.select(pv(lc[B]), bh2, pv(hc[A]), pv(lc[B]))
                        nc.vector.select(pv(hc[B]), bh2, pv(tcnt), pv(hc[B]))
                        nc.vector.select(pv(lc[B]), bl2, pv(tcnt), pv(lc[B]))
                        nc.vector.select(pv(hc[B]), bl2, pv(lc[A]), pv(hc[B]))
                    nc.vector.tensor_scalar(tmp4[:], chieff[A][:], 0.5,
                                            scalar2=float(K) * 0.5,
                                            op0=Alu.mult, op1=Alu.add)
                    nc.vector.select(chieff[B][:], mge[:], hc[B][:, NCH:32],
                                     tmp4[:])
                    if repair:
                        nc.vector.tensor_tensor(mok[:], mbh[:], mbl[:],
                                                op=Alu.logical_or)
                        nc.vector.select(chieff[B][:], mok[:],
                                         hc[B][:, NCH:32], chieff[B][:])

                FI = N_ROUNDS % 2
                hiF = hc[FI][:, 0:NCH]
                chiF = hc[FI][:, NCH:32]
                # ------- finisher + final, interleaved by halves -------
                # chi (exact count at hi) is tracked through the rounds, so
                # no recount pass. W = (d2 <= hi) * d2 in one stt (zeros
                # lose max8 to any real d2 > 0). Tfin for each half of 8
                # chunks is resolved as soon as its max8s land, so ACT exp
                # and output DMA of half 0 overlap the finisher of half 1.
                # A = sim * rowmask only; the directed-knn symmetrization
                # C = max(A, A^T) runs on the host (sim is symmetric, >= 0,
                # so max(A, A^T) == sim * max(R, R^T) exactly).
                with tc.tile_pool(name="fsim", bufs=3) as fsim, \
                     tc.tile_pool(name="fmsk", bufs=2) as fmsk, \
                     tc.tile_pool(name="fout", bufs=2) as fout:
                    for half in range(2):
                        hs = slice(half * 8, half * 8 + 8)
                        h64 = slice(half * 64, half * 64 + 64)
                        for c in range(half * 8, half * 8 + 8):
                            d2c = D2[:, c * N:(c + 1) * N]
                            wscr = rscrA.tile([128, N], F32, tag="rscrA",
                                              name="wsd")
                            nc.vector.scalar_tensor_tensor(
                                out=wscr[:], in0=d2c, scalar=hiF[:, c:c + 1],
                                in1=d2c, op0=Alu.is_le, op1=Alu.mult)
                            nc.vector.max(out=w8[:, c * 8:(c + 1) * 8],
                                          in_=wscr[:])
                        # j = chi - K for this half
                        nc.vector.tensor_scalar(tmp1[:, hs], chiF[:, hs],
                                                float(K), scalar2=None,
                                                op0=Alu.subtract)
                        nc.vector.tensor_scalar(mge[:, hs], tmp1[:, hs], 0.0,
                                                scalar2=None, op0=Alu.is_ge)
                        nc.vector.tensor_scalar(mbh[:, hs], tmp1[:, hs], 7.0,
                                                scalar2=None, op0=Alu.is_le)
                        nc.vector.tensor_tensor(mok[:, hs], mge[:, hs],
                                                mbh[:, hs],
                                                op=Alu.logical_and)
                        nc.vector.tensor_tensor(
                            ohsel[:, h64].rearrange("p (c i) -> p c i", i=8),
                            iota8f[:, h64].rearrange("p (c i) -> p c i", i=8),
                            tmp1[:, hs].unsqueeze(2).to_broadcast(
                                [128, 8, 8]),
                            op=Alu.is_equal)
                        nc.vector.tensor_mul(ohsel[:, h64], ohsel[:, h64],
                                             w8[:, h64])
                        nc.vector.tensor_reduce(
                            tmp3[:, hs],
                            ohsel[:, h64].rearrange("p (c i) -> p c i", i=8),
                            axis=X_AX, op=Alu.add)
                        # fallback: j>7 -> w8[7] (rank chi-7); j<0 -> hi
                        w87 = w8[:, h64].rearrange(
                            "p (c i) -> p c i", i=8)[:, :, 7:8].squeeze(2)
                        nc.vector.select(tmp4[:, hs], mge[:, hs], w87,
                                         hiF[:, hs])
                        nc.vector.select(Tfin[:, hs], mok[:, hs],
                                         tmp3[:, hs], tmp4[:, hs])
                        # final phase for this half
                        for c in range(half * 8, half * 8 + 8):
                            d2c = D2[:, c * N:(c + 1) * N]
                            simt = fsim.tile([128, N], BF16, tag="simt",
                                             name="simt")
                            nc.scalar.activation(simt[:], d2c, Act.Exp,
                                                 bias=0.0, scale=neginvb[:])
                            ms = fmsk.tile([128, N], BF16, tag="ms",
                                           name="ms")
                            nc.vector.tensor_scalar(
                                ms[:], d2c, Tfin[:, c:c + 1], scalar2=None,
                                op0=Alu.is_le)
                            ot = fout.tile([128, N], BF16, tag="ot",
                                           name="ot")
                            nc.vector.tensor_mul(ot[:], ms[:], simt[:])
                            if c % 2 == 0:
                                nc.sync.dma_start(
                                    outp[c * 128:(c + 1) * 128, :], ot[:])
                            else:
                                nc.gpsimd.dma_start(
                                    outp[c * 128:(c + 1) * 128, :], ot[:])
    nc.compile()
    return nc


_NC_CACHE = None
LAST_RESULTS = None


def _get_nc():
    global _NC_CACHE
    if _NC_CACHE is None:
        _NC_CACHE = build_nc()
    return _NC_CACHE


def kernel(x, W):
    from concourse.bass_utils import run_bass_kernel_spmd
    x = np.ascontiguousarray(np.asarray(x, dtype=np.float32))
    W = np.ascontiguousarray(np.asarray(W, dtype=np.float32))
    nc = _get_nc()
    in_maps = []
    for i in range(8):
        h, b = i // 2, i % 2
        in_maps.append({"xb": np.ascontiguousarray(x[b]),
                        "wh": np.ascontiguousarray(W[h])})
    res = run_bass_kernel_spmd(nc, in_maps, core_ids=list(range(8)))
    global LAST_RESULTS
    LAST_RESULTS = res
    C = [np.asarray(res.results[i]["outp"], dtype=np.float32)
         for i in range(8)]
    # symmetrize the directed knn masks on host: C_h = max(A_h, A_h^T)
    C = [np.maximum(a, a.T) for a in C]
    adj = np.stack([
        (C[0 + b] + C[2 + b] + C[4 + b] + C[6 + b]) * 0.25 for b in range(2)
    ]).astype(np.float32)
    return adj

